# revision 53
# baseline (speedup 1.0000x reference)
"""Multi-head self-attention (B=4, S=2048, E=1024, H=16) on 8 NeuronCores.

Sharding: batch (4) x head-group (2 groups of 8 heads), one (b, g) pair per
core.  Each core computes Q/K/V projections for its head group, attention,
and a partial output projection (row-parallel over Wo); the host sums the
two head-group partials per batch.

Layout strategy: the host feeds x transposed (xT = x.T, [E, S]) so every
matmul's contraction dim lands on SBUF partitions with no on-chip
transposes.  Scores are computed transposed (scoresT[k, q] = K @ Q^T per
head, two heads packed into disjoint PE row groups), softmax denominators
come free via a ones-column appended to V (attn @ [V|1] with V stationary
directly yields attnT[d, q] plus the denom row - exactly the lhsT the
output projection needs).

All matmul operands are fp16 (2x PE column rate vs f32r; fp32 PSUM
accumulation).  The schedule is a software-pipelined global tick loop
over the 8 (chunk, head-pair) attention calls: call i's scores occupy
ticks 16i..16i+15 and its AV accumulation lags by a staggered skew, so
ScalarE (exp, the critical engine at ~2.2us/tick) never idles at call
boundaries.  Projection and chunk-0 output-projection groups are hosted
one per tick inside the stream; groups inside reciprocal-carrying
regions release their PSUM via ScalarE (Identity+bias) because the Tile
scheduler orders engine queues by simulated readiness and would
otherwise park their Vector bias-add behind 13us of reciprocals.
Softmax normalization is split: a quick PSUM->SBUF copy releases the av
banks immediately; the exact DVE reciprocal+broadcast+mul run ~4 ticks
later, off the critical path (reciprocal_approx_fast mis-executes on HW,
Scalar FT.Reciprocal is blocked for accuracy, and GpSimd cannot access
PSUM - its queue is also blocked >10us by tile-release pseudo
instructions at the chunk transition, where a DRAM-bounce broadcast is
used instead).
"""

import numpy as np

import concourse.bacc as bacc
import concourse.mybir as mybir
import concourse.tile as tile
from concourse.bass_utils import run_bass_kernel_spmd

B, S, E, H = 4, 2048, 1024, 16
GROUPS = 2                 # tensor-parallel head groups
HG = H // GROUPS           # heads per core
DH = E // H                # head dim
DG = HG * DH               # projected dim per core (512)
ET, DT, ST = E // 128, DG // 128, S // 128
QCH = 1024                 # q-chunk (psum tile free size, 2 banks)
NQC = S // QCH
NH = QCH // 512            # N=512 matmul halves per chunk
SCALE = 1.0 / np.sqrt(DH)

f32 = mybir.dt.float32
f16 = mybir.dt.float16
FT = mybir.ActivationFunctionType

_CACHE = {}


def _body(nc, tc, xT, wq, wk, wv, wo, bqk, bv, bo, out):
    with tc.tile_pool(name="pers", bufs=1) as pers, \
         tc.tile_pool(name="pp", bufs=1, space="PSUM") as pp:
        # Per-block tiles (not one big tile): Tile dependency tracking is
        # per-tile, so a single kt/qt/vv tile would make every attention
        # call's first scores matmul false-depend on the LATEST hosted
        # projection write - which queues behind the norm reciprocals on
        # Vector and stalls the PE ~10us at every call boundary.
        qt = [[pers.tile([128, QCH], f16, name=f"qt{m}_{c}")
               for c in range(NQC)] for m in range(DT)]
        kt = [[pers.tile([128, QCH], f16, name=f"kt{m}_{c}")
               for c in range(NQC)] for m in range(DT)]
        vv = [pers.tile([128, HG, DH + 1], f16, name=f"vv{s}")
              for s in range(ST)]
        for s in range(ST):
            nc.gpsimd.memset(vv[s][:, :, DH:DH + 1], 1.0)
        rcd = nc.dram_tensor("rcd", [16, QCH], f32, kind="Internal").ap()
        bqk_sb = pers.tile([128, 2 * DT], f32)
        nc.sync.dma_start(out=bqk_sb, in_=bqk)

        # ---- input / weight DMAs: ONE descriptor per tensor - the Sync
        # queue serializes issue at ~0.7us/descriptor, so many small DMAs
        # delay the whole startup more than one large transfer does.  Order
        # tracks first consumption: K(m0,c0) -> Q(m0,c0) -> chunk 1 -> V.
        wk_sb = pers.tile([128, ET, DG], f16, name="wk_sb")
        wq_sb = pers.tile([128, ET, DG], f16, name="wq_sb")
        wv_sb = pers.tile([128, ET, DG], f16, name="wv_sb")
        xts = [pers.tile([128, ET, QCH], f16, name=f"x{c}")
               for c in range(NQC)]
        nc.sync.dma_start(out=wk_sb,
                          in_=wk.rearrange("(a p) d -> p a d", p=128))
        nc.sync.dma_start(
            out=xts[0],
            in_=xT[:, 0:QCH].rearrange("(a p) q -> p a q", p=128))
        nc.sync.dma_start(out=wq_sb,
                          in_=wq.rearrange("(a p) d -> p a d", p=128))
        nc.sync.dma_start(
            out=xts[1],
            in_=xT[:, QCH:S].rearrange("(a p) q -> p a q", p=128))
        nc.sync.dma_start(out=wv_sb,
                          in_=wv.rearrange("(a p) d -> p a d", p=128))
        bvbc = pers.tile([128, DG], f32)              # bv broadcast over s
        nc.sync.dma_start(out=bvbc, in_=bv.to_broadcast((128, DG)))
        wo_sb = pers.tile([128, DT, E], f16)
        nc.sync.dma_start(
            out=wo_sb, in_=wo.rearrange("(a p) e -> p a e", p=128)
        )
        bobc = pers.tile([128, E], f32)               # bo broadcast over s
        nc.sync.dma_start(out=bobc, in_=bo.to_broadcast((128, E)))

        # ---- projection work groups (emitted up-front or via extra hooks)
        def qk_proj(wsb, dst, ip, m, c, eng="v"):
            ps = pp.tile([128, QCH], f32, tag="mm", bufs=2, name="ps_qk")
            for k in range(ET):
                for nn in range(NH):
                    nc.tensor.matmul(
                        ps[:, nn * 512:(nn + 1) * 512],
                        wsb[:, k, m * 128:(m + 1) * 128],
                        xts[c][:, k, nn * 512:(nn + 1) * 512],
                        start=(k == 0),
                        stop=(k == ET - 1),
                    )
            # eng="s": biased PSUM->SBUF move on ScalarE.  Used for groups
            # hosted inside calls that carry deferred reciprocals: the Tile
            # scheduler orders each engine queue by simulated readiness, so
            # a Vector bias-add whose PSUM inputs are still accumulating
            # gets queued BEHIND the 13us of reciprocals - and the "mm"
            # PSUM buffer it releases then stalls the PE's next scores.
            with nc.allow_low_precision(reason="fp16 activations for PE"):
                bias = bqk_sb[:, ip * DT + m:ip * DT + m + 1]
                dst_ap = dst[m][c]
                if eng == "s":
                    nc.scalar.activation(out=dst_ap, in_=ps,
                                         func=FT.Identity, bias=bias)
                else:
                    nc.vector.tensor_scalar_add(dst_ap, ps, bias)

        def v_proj(ms):
            ps = pp.tile([128, QCH], f32, tag="mm", bufs=2, name="ps_v")
            for k in range(ET):
                nc.tensor.matmul(
                    ps[:, 0:DG],
                    xts[ms // (ST // NQC)][
                        :, k,
                        (ms % (ST // NQC)) * 128:(ms % (ST // NQC) + 1) * 128],
                    wv_sb[:, k, :],
                    start=(k == 0),
                    stop=(k == ET - 1),
                )
            with nc.allow_low_precision(reason="fp16 V for PE"):
                nc.vector.tensor_add(
                    vv[ms][:, :, 0:DH],
                    ps[:, 0:DG].rearrange("p (h d) -> p h d", h=HG),
                    bvbc.rearrange("p (h d) -> p h d", h=HG),
                )

        with tc.tile_pool(name="p3", bufs=1) as p3:
            at = [None, None]  # at[chunk] -> list of DT attnT tiles

            def outproj_ms(c, ms):
                po = pp.tile([128, E], f32, tag="mm", bufs=2, name="po")
                for j in range(DT):
                    for nn in range(E // 512):
                        lo, hi = nn * 512, (nn + 1) * 512
                        nc.tensor.matmul(
                            po[:, lo:hi],
                            at[c][j][:, ms * 128:(ms + 1) * 128],
                            wo_sb[:, j, lo:hi],
                            start=(j == 0),
                            stop=(j == DT - 1),
                        )
                ou = p3.tile([128, E], f32, tag="out", bufs=3, name="ou")
                nc.vector.tensor_add(ou, po, bobc)
                r0 = c * QCH + ms * 128
                nc.sync.dma_start(out=out[r0:r0 + 128, :], in_=ou)

            at[0] = [p3.tile([128, QCH], f16, tag="attnT", bufs=2 * DT,
                             name=f"at{j}") for j in range(DT)]
            at[1] = [p3.tile([128, QCH], f16, tag="attnT", bufs=2 * DT,
                             name=f"bt{j}") for j in range(DT)]

            def emit_scores(i, kk, exq):
                c, pr = CALLS[i]
                scs = [pp.tile([128, QCH], f32, tag="mm", bufs=2,
                               name=f"sc{x}") for x in range(2)]
                for x, sc in enumerate(scs):
                    o = x * 64
                    for nn in range(NH):
                        lo, hi = nn * 512, (nn + 1) * 512
                        nc.tensor.matmul(
                            sc[:, lo:hi],
                            kt[pr][kk // (ST // NQC)][
                                o:o + 64,
                                (kk % (ST // NQC)) * 128:
                                (kk % (ST // NQC) + 1) * 128],
                            qt[pr][c][o:o + 64, lo:hi],
                            start=True,
                            stop=True,
                        )
                    ex = p3.tile([128, QCH], f16, tag="expt", bufs=14,
                                 name="ex")
                    nc.scalar.activation(out=ex, in_=sc, func=FT.Exp,
                                         scale=SCALE)
                    exq[(i, kk, x)] = ex

            def emit_avs(i, k2, avs, exq):
                c, pr = CALLS[i]
                for x, av in enumerate(avs):
                    h = 2 * pr + x
                    ex = exq.pop((i, k2, x))
                    for nn in range(NH):
                        lo, hi = nn * 512, (nn + 1) * 512
                        nc.tensor.matmul(
                            av[0:DH + 1, lo:hi],
                            vv[k2][:, h],
                            ex[:, lo:hi],
                            start=(k2 == 0),
                            stop=(k2 == ST - 1),
                        )

            # ---- software-pipelined global schedule -----------------------
            # One continuous tick stream over all 8 (chunk, head-pair)
            # calls: call i's scores occupy global ticks 16i..16i+15, its
            # AV lags by SKEWS[i], so the next call's scores/exps flow
            # during the previous call's AV drain and ScalarE (exp - the
            # critical engine) never idles at call boundaries.  Skews are
            # staggered so per-call AV emission windows never overlap
            # (the "av" PSUM pair is reused by consecutive calls).
            K, Q, V, OP = qk_proj, qk_proj, v_proj, outproj_ms
            CALLS = [(c, pr) for c in range(NQC) for pr in range(HG // 2)]
            SKEWS = [6, 5, 4, 3, 3, 3, 3, 3]
            NT = 16
            workq = {}

            def sched(g, fn):
                workq.setdefault(g, []).append(fn)

            # hosted projection groups (emitted at their global tick; "s"
            # variants release their PSUM via ScalarE - see qk_proj)
            sched(0, lambda: K(wk_sb, kt, 1, 1, 0))
            sched(1, lambda: Q(wq_sb, qt, 0, 1, 0))
            sched(2, lambda: K(wk_sb, kt, 1, 0, 1))
            for ms in range(ST):
                sched(3 + ms, lambda ms=ms: V(ms))
            sched(19, lambda: K(wk_sb, kt, 1, 1, 1))
            sched(20, lambda: K(wk_sb, kt, 1, 2, 0, "s"))
            sched(22, lambda: K(wk_sb, kt, 1, 2, 1, "s"))
            sched(24, lambda: Q(wq_sb, qt, 0, 2, 0, "s"))
            sched(32, lambda: K(wk_sb, kt, 1, 3, 0, "s"))
            sched(34, lambda: K(wk_sb, kt, 1, 3, 1, "s"))
            sched(36, lambda: Q(wq_sb, qt, 0, 3, 0, "s"))
            sched(48, lambda: Q(wq_sb, qt, 0, 0, 1, "s"))
            sched(50, lambda: Q(wq_sb, qt, 0, 1, 1, "s"))
            sched(52, lambda: Q(wq_sb, qt, 0, 2, 1, "s"))
            sched(64, lambda: Q(wq_sb, qt, 0, 3, 1, "s"))
            sched(72, lambda: OP(0, 0))
            sched(74, lambda: OP(0, 1))
            sched(86, lambda: OP(0, 2))
            sched(88, lambda: OP(0, 3))
            sched(102, lambda: OP(0, 4))
            sched(104, lambda: OP(0, 5))
            sched(118, lambda: OP(0, 6))
            sched(120, lambda: OP(0, 7))

            # up-front: just what the first two score ticks need
            qk_proj(wk_sb, kt, 1, 0, 0)
            qk_proj(wq_sb, qt, 0, 0, 0)

            exq = {}
            call_avs = [None] * len(CALLS)
            G = NT * len(CALLS) + SKEWS[-1] + 1
            for g in range(G + 8):
                i = g // NT
                if i < len(CALLS) and g % NT == 0:
                    call_avs[i] = [pp.tile([128, QCH], f32, tag="av",
                                           bufs=2, name=f"av{x}")
                                   for x in range(2)]
                if i < len(CALLS):
                    emit_scores(i, g % NT, exq)
                for w in workq.pop(g, ()):
                    w()
                # lagging AVs, oldest call first
                for j in range(max(0, i - 1), min(i + 1, len(CALLS) - 1) + 1):
                    if j >= len(CALLS):
                        continue
                    k2 = g - NT * j - SKEWS[j]
                    if 0 <= k2 < ST:
                        emit_avs(j, k2, call_avs[j], exq)
                        if k2 == ST - 1:
                            # call j fully accumulated: quick-release its
                            # PSUM now, normalize ~4 ticks later
                            c, pr = CALLS[j]
                            avcs = attention_copy(call_avs[j])
                            if j < len(CALLS) - 1:
                                bnc = (c, pr) == (0, 3)
                                sched(g + 4,
                                      lambda a=avcs, c=c, pr=pr, b=bnc:
                                      attention_finish(a, c, pr, bounce=b))
                            else:
                                last_avcs = (avcs, c, pr)

            # tail: start two output tiles' j=0..2 accumulation before the
            # final finish (they only need at[1][0..2]), complete with j=3
            # once the last normalization lands
            def op1_partial(ms, tag="mm"):
                po = pp.tile([128, E], f32, tag=tag, bufs=2, name="po")
                for j in range(DT - 1):
                    for nn in range(E // 512):
                        lo, hi = nn * 512, (nn + 1) * 512
                        nc.tensor.matmul(
                            po[:, lo:hi],
                            at[1][j][:, ms * 128:(ms + 1) * 128],
                            wo_sb[:, j, lo:hi],
                            start=(j == 0),
                            stop=False,
                        )
                return po

            def op1_complete(ms, po):
                j = DT - 1
                for nn in range(E // 512):
                    lo, hi = nn * 512, (nn + 1) * 512
                    nc.tensor.matmul(
                        po[:, lo:hi],
                        at[1][j][:, ms * 128:(ms + 1) * 128],
                        wo_sb[:, j, lo:hi],
                        start=False,
                        stop=True,
                    )
                ou = p3.tile([128, E], f32, tag="out", bufs=3, name="ou")
                nc.vector.tensor_add(ou, po, bobc)
                r0 = QCH + ms * 128
                nc.sync.dma_start(out=out[r0:r0 + 128, :], in_=ou)

            # four partials: ms0/1 on the "mm" rotation, ms2/3 on the "av"
            # banks (free once the last call's quick-release copies run)
            pos = [op1_partial(ms) for ms in (0, 1)]
            pos += [op1_partial(ms, tag="av") for ms in (2, 3)]
            attention_finish(*last_avcs, fast=True, bounce=True)
            for ms, po in zip((0, 1, 2, 3), pos):
                op1_complete(ms, po)
            for ms in range(4, QCH // 128):
                outproj_ms(1, ms)


def _declare(nc):
    xT = nc.dram_tensor("xT", [E, S], f16, kind="ExternalInput").ap()
    wq = nc.dram_tensor("wq", [E, DG], f16, kind="ExternalInput").ap()
    wk = nc.dram_tensor("wk", [E, DG], f16, kind="ExternalInput").ap()
    wv = nc.dram_tensor("wv", [E, DG], f16, kind="ExternalInput").ap()
    wo = nc.dram_tensor("wo", [DG, E], f16, kind="ExternalInput").ap()
    bqk = nc.dram_tensor("bqk", [128, 2 * DT], f32, kind="ExternalInput").ap()
    bv = nc.dram_tensor("bv", [1, DG], f32, kind="ExternalInput").ap()
    bo = nc.dram_tensor("bo", [1, E], f32, kind="ExternalInput").ap()
    out = nc.dram_tensor("out", [S, E], f32, kind="ExternalOutput").ap()
    return xT, wq, wk, wv, wo, bqk, bv, bo, out


def _build():
    nc = bacc.Bacc("TRN2", target_bir_lowering=False, debug=False)
    args = _declare(nc)
    with tile.TileContext(nc) as tc:
        _body(nc, tc, *args)
    nc.compile()
    return nc


def _in_maps(inputs):
    x = np.asarray(inputs["inputs"], np.float32)
    maps = []
    for b in range(B):
        xT = np.ascontiguousarray(x[b].T).astype(np.float16)
        for g in range(GROUPS):
            sl = slice(g * DG, (g + 1) * DG)
            bq_g = np.asarray(inputs["bq"], np.float32)[sl]
            bk_g = np.asarray(inputs["bk"], np.float32)[sl]
            bqk = np.concatenate(
                [bq_g.reshape(DT, 128).T, bk_g.reshape(DT, 128).T], axis=1
            )
            maps.append({
                "xT": xT,
                "wq": np.asarray(inputs["Wq"], np.float32)[:, sl].astype(np.float16),
                "wk": np.asarray(inputs["Wk"], np.float32)[:, sl].astype(np.float16),
                "wv": np.asarray(inputs["Wv"], np.float32)[:, sl].astype(np.float16),
                "wo": np.asarray(inputs["Wo"], np.float32)[sl, :].astype(np.float16),
                "bqk": np.ascontiguousarray(bqk),
                "bv": np.asarray(inputs["bv"], np.float32)[sl].reshape(1, DG),
                "bo": np.asarray(inputs["bo"], np.float32).reshape(1, E),
            })
    return maps


def kernel(**inputs) -> np.ndarray:
    if "nc" not in _CACHE:
        _CACHE["nc"] = _build()
    nc = _CACHE["nc"]
    res = run_bass_kernel_spmd(nc, _in_maps(inputs), core_ids=list(range(B * GROUPS)))
    out = np.zeros((B, S, E), np.float32)
    for b in range(B):
        out[b] = res.results[2 * b]["out"] + res.results[2 * b + 1]["out"]
    return out
